# revision 23
# baseline (speedup 1.0000x reference)
"""AngProtoLoss (stable) distributed Bass kernel for 8 TRN2 NeuronCores.

Problem (reference):
    dvecs: (4096, 16, 512) f32
    centroids = mean(dvecs, axis=1)                  # (N, D)
    u = dvecs[:, -1, :]                              # (N, D)
    cos = clip(cos_sim(u, centroids), min=1e-6)      # (N, N)
    logits = cos * w + b
    loss = -mean(diag(log_softmax(logits)))
        = mean_i [ logsumexp_k(w*clip(cos_ik)) - w*clip(cos_ii) ]   (b cancels)

Sharding: data-parallel over speakers N; 512 speakers (4 chunks of 128) per
core.

Key structure (v2 — PE-centric):
 - The m-sum (centroid numerator) rides the TensorEngine: 16 accumulating
   float32r matmuls per chunk against a bitcast identity (f32r streams at
   1 cycle/row for free-dim >= 256), instead of a DVE add-tree. This frees
   the DVE (the v1 bottleneck: 100% busy for 45us) and keeps the PE warm.
 - Centroids are normalized (rs_c) before the fp8 transpose+allgather.
   u is NOT normalized: rs_u is folded into the phase-C epilogue as
   per-partition tensor_scalar operands: y = max(ps*rs_u, eps).
 - Per-chunk fp8 AllGather fires as soon as that chunk's cT is ready
   (~2us after its load lands), pipelining the AG chain against the
   remaining loads. Bounce writes + AGs + gathered reads all ride the
   otherwise-empty gpsimd SWDGE ring so they are never queued behind the
   16.8MB of X-load descriptors on the sync HWDGE ring.
 - Phase-C matmul groups are interleaved into the PE FIFO in
   expected-data-arrival order (g0 after chunk2's m-sum, g1 split around
   chunk3's m-sum) so the PE never head-of-line blocks the AG-critical
   m-sums, yet fills its DMA-wait gaps with useful work.
 - Device ships per-(chunk, gather-group) exp-sums and the diagonal cos;
   host does s = sum(parts), rows = log(s) - w*clip(diag), mean.
"""

import os
import sys

for _p in ("/opt/trn_rl_repo",):
    if os.path.isdir(_p) and _p not in sys.path:
        sys.path.append(_p)

import numpy as np

import concourse.bass as bass
import concourse.tile as tile
from concourse import bacc, mybir
from concourse.bass_utils import run_bass_kernel_spmd
from concourse.masks import make_identity

N_CORES = 8
N, M, D = 4096, 16, 512
P = 128                     # partitions
LOCAL = N // N_CORES        # 512 speakers per core
NCHUNK = LOCAL // P         # 4 chunks of 128 speakers
NT = D // P                 # 4 d-tiles
EPS = 1e-6

F32 = mybir.dt.float32
F32R = mybir.dt.float32r
BF16 = mybir.dt.bfloat16
FP8 = mybir.dt.float8e4
AF = mybir.ActivationFunctionType
ALU = mybir.AluOpType


def build_program(w_val: float):
    nc = bacc.Bacc("TRN2", target_bir_lowering=False, debug=False,
                   num_devices=N_CORES)
    # f32r is bit-identical to f32; typing the input chain as f32r lets the
    # m-sum matmuls stream at 1 cycle/row (the verifier requires an all-f32r
    # producer chain for f32r matmul inputs).
    dvecs = nc.dram_tensor("dvecs", [LOCAL, M, D], F32R, kind="ExternalInput").ap()
    out = nc.dram_tensor("out", [5, LOCAL], F32, kind="ExternalOutput").ap()

    with tile.TileContext(nc) as tc:
        _build(nc, tc, dvecs, out, w_val)
    nc.compile()
    return nc


def _build(nc, tc, dvecs, out, w_val):
    from contextlib import ExitStack
    ctx = ExitStack()
    with ctx:
        singles = ctx.enter_context(tc.tile_pool(name="singles", bufs=1))
        xpool = ctx.enter_context(tc.tile_pool(name="xpool", bufs=3))
        cpool = ctx.enter_context(tc.tile_pool(name="cpool", bufs=2))
        scr = ctx.enter_context(tc.tile_pool(name="scr", bufs=3))
        gpool = ctx.enter_context(tc.tile_pool(name="gpool", bufs=1))
        epool = ctx.enter_context(tc.tile_pool(name="epool", bufs=3))
        cpsum = ctx.enter_context(tc.tile_pool(name="cpsum", bufs=2, space="PSUM"))
        tpsum = ctx.enter_context(tc.tile_pool(name="tpsum", bufs=2, space="PSUM"))
        mpsum = ctx.enter_context(tc.tile_pool(name="mpsum", bufs=2, space="PSUM"))
        dram = ctx.enter_context(tc.tile_pool(name="dram", bufs=1, space="DRAM"))

        ident = singles.tile([P, P], F32)
        make_identity(nc, ident)
        ident_bf = singles.tile([P, P], BF16)
        make_identity(nc, ident_bf)
        # f32r identity: the verifier wants f32r matmul inputs produced by a
        # rounding op, so copy the f32 identity through the ACT engine.
        ident_r = singles.tile([P, P], F32R)
        nc.scalar.copy(ident_r, ident)

        # persistent across the whole kernel
        uT = singles.tile([P, NT, LOCAL], BF16)       # u^T (raw): [d, t, i]
        ssq = singles.tile([P, NCHUNK, 2], F32)       # |c|^2, |u|^2
        nrm = singles.tile([P, NCHUNK, 2], F32)       # |c|, |u|
        rs = singles.tile([P, NCHUNK, 2], F32)        # 1/|c|, 1/|u|
        # stats[:, 0:4, q] = partial exp-sums per gather group
        # stats[:, 4, q]  = diag cos
        stats = singles.tile([P, 5, NCHUNK], F32)

        # ---------- phase A: all X loads queue first on the sync ring ----
        xs = []
        for r in range(NCHUNK):
            x = xpool.tile([P, M, D], F32R, name=f"x{r}", tag="x")
            if r == 0:
                # fine-grained so chunk0's m-sum can trail the DMA and its
                # allgather fires earliest
                for j in range(M // 2):
                    nc.sync.dma_start(
                        out=x[:, 2 * j:2 * j + 2, :],
                        in_=dvecs[0:P, 2 * j:2 * j + 2, :])
            else:
                nc.sync.dma_start(out=x, in_=dvecs[r * P:(r + 1) * P, :, :])
            xs.append(x)

        gath = [None] * NCHUNK
        g_sb = [None] * NCHUNK

        # ---------- phase B: per-chunk centroid pipeline + allgather -----
        def emit_chunk(r):
            x = xs[r]
            # m-sum on the PE: csum[i, d] = sum_m x[i, m, d] (f32r @ 1cyc/row)
            csum = cpsum.tile([P, D], F32, name=f"csum{r}", tag="csum")
            for m in range(M):
                nc.tensor.matmul(csum, ident_r, x[:, m, :],
                                 start=(m == 0), stop=(m == M - 1))
            # norms: ssq via ACT Square+accum; sqrt; reciprocal on DVE
            sq_c = scr.tile([P, D], BF16, name=f"sqc{r}", tag="sq")
            nc.scalar.activation(sq_c, csum, AF.Square,
                                 accum_out=ssq[:, r, 0:1])
            sq_u = scr.tile([P, D], BF16, name=f"squ{r}", tag="sq")
            nc.scalar.activation(sq_u, x[:, M - 1, :].bitcast(F32), AF.Square,
                                 accum_out=ssq[:, r, 1:2])
            nc.scalar.activation(nrm[:, r, :], ssq[:, r, :], AF.Sqrt)
            nc.vector.reciprocal(rs[:, r, :], nrm[:, r, :])
            # normalized centroid + last utterance (bf16)
            chat = cpool.tile([P, D], BF16, name=f"chat{r}", tag="chat")
            nc.vector.tensor_scalar_mul(chat, csum, rs[:, r, 0:1])
            uhat = cpool.tile([P, D], BF16, name=f"uhat{r}", tag="uhat")
            nc.vector.tensor_scalar_mul(uhat, x[:, M - 1, :].bitcast(F32),
                                        rs[:, r, 1:2])
            # diag cos = sum_d chat*uhat
            dscr = scr.tile([P, D], BF16, name=f"dscr{r}", tag="sq")
            nc.vector.tensor_mul(dscr, chat, uhat)
            nc.vector.tensor_reduce(stats[:, 4, r:r + 1], dscr,
                                    axis=mybir.AxisListType.X,
                                    op=ALU.add)
            # uhat transposes -> one PSUM bank -> uT bf16 (single eviction)
            pu = tpsum.tile([P, NT, P], BF16, name=f"ptu{r}", tag="pt")
            for t in range(NT):
                nc.tensor.transpose(pu[:, t, :], uhat[:, t * P:(t + 1) * P],
                                    ident_bf)
            nc.scalar.copy(uT[:, :, r * P:(r + 1) * P], pu)
            # chat transposes (bf16) -> one PSUM bank -> cT fp8
            cT = cpool.tile([P, NT, P], FP8, name=f"cT{r}", tag="cT")
            pc = tpsum.tile([P, NT, P], BF16, name=f"ptc{r}", tag="pt")
            for t in range(NT):
                nc.tensor.transpose(pc[:, t, :], chat[:, t * P:(t + 1) * P],
                                    ident_bf)
            nc.scalar.copy(cT, pc)
            # bounce + allgather on the gpsimd SWDGE ring
            bounce = dram.tile([P, NT * P], FP8, name=f"bounce{r}")
            nc.gpsimd.dma_start(out=bounce,
                                in_=cT.rearrange("p t i -> p (t i)"))
            g = dram.tile([N_CORES * P, NT * P], FP8, name=f"gath{r}",
                          addr_space="Shared")
            nc.gpsimd.collective_compute(
                "AllGather", ALU.bypass,
                replica_groups=[list(range(N_CORES))],
                ins=[bounce.opt()], outs=[g.opt()],
            )
            gath[r] = g

        def emit_gread(gi):
            # gathered fp8 centroids -> SBUF [d, rank, t, i]. Rides the
            # scalar engine's HWDGE ring (qActDynamicHW): separate from the
            # sync ring so it is never queued behind the 16.8MB of X-load
            # descriptors, and HWDGE (not SWDGE) for the Shared-space read.
            g_sb[gi] = gpool.tile([P, N_CORES, NT, P], FP8, name=f"gsb{gi}",
                                  tag=f"gsb{gi}")
            nc.scalar.dma_start(
                out=g_sb[gi],
                in_=gath[gi].rearrange("(c p) f -> p c f", p=P).rearrange(
                    "p c (t i) -> p c t i", t=NT))

        def emit_cq(gi, q):
            # 512 queries x 1024 columns for gather group gi, query chunk q
            ps = mpsum.tile([P, 2, 512], F32, name=f"ps{gi}_{q}", tag="ps")
            for h in range(2):
                for t in range(NT):
                    nc.tensor.matmul(
                        ps[:, h, :],
                        uT[:, t, q * P:(q + 1) * P],
                        g_sb[gi][:, 4 * h:4 * h + 4, t, :],
                        start=(t == 0), stop=(t == NT - 1),
                    )
            # y = max(cos, eps); s_part = sum_k exp(w*y)
            y = epool.tile([P, 2 * 512], BF16, name=f"y{gi}_{q}", tag="y")
            nc.vector.tensor_scalar_max(y, ps.rearrange("p a b -> p (a b)"),
                                        EPS)
            e_scr = epool.tile([P, 2 * 512], BF16, name=f"e{gi}_{q}", tag="e")
            nc.scalar.activation(e_scr, y, AF.Exp, scale=w_val,
                                 accum_out=stats[:, gi, q:q + 1])

        # ALL phase-B work first on every engine FIFO, phase C strictly
        # after: kernel-start skew across ranks (15-45us observed) makes
        # gather completion times unpredictable, and any phase-B op queued
        # behind a gather-dependent op cascades into a late allgather.
        # With this ordering every bounce/AG fires on load cadence no
        # matter how late the collectives run.
        for r in range(NCHUNK):
            emit_chunk(r)
        for gi in range(NCHUNK):
            emit_gread(gi)
            for q in range(NCHUNK):
                emit_cq(gi, q)

        # ---------- ship results (per-row writes, baseline pattern) ------
        for k in range(5):
            nc.sync.dma_start(out=out[k].rearrange("(q p) -> p q", p=P),
                              in_=stats[:, k, :])


_CACHE = {}


def kernel(dvecs, w, b):
    w_val = float(np.asarray(w))
    key = w_val
    if key not in _CACHE:
        _CACHE[key] = build_program(w_val)
    nc = _CACHE[key]
    dvecs = np.ascontiguousarray(np.asarray(dvecs, dtype=np.float32))
    in_maps = [
        {"dvecs": dvecs[c * LOCAL:(c + 1) * LOCAL]} for c in range(N_CORES)
    ]
    res = run_bass_kernel_spmd(nc, in_maps, core_ids=list(range(N_CORES)))
    total = 0.0
    for c in range(N_CORES):
        o = np.asarray(res.results[c]["out"], dtype=np.float64)
        s = o[0:4].sum(axis=0)
        diag = o[4]
        rows = np.log(s) - w_val * np.maximum(diag, EPS)
        total += float(rows.sum())
    return np.float32(total / N)


# revision 24
# speedup vs baseline: 1.0091x; 1.0091x over previous
"""AngProtoLoss (stable) distributed Bass kernel for 8 TRN2 NeuronCores.

Problem (reference):
    dvecs: (4096, 16, 512) f32
    centroids = mean(dvecs, axis=1)                  # (N, D)
    u = dvecs[:, -1, :]                              # (N, D)
    cos = clip(cos_sim(u, centroids), min=1e-6)      # (N, N)
    logits = cos * w + b
    loss = -mean(diag(log_softmax(logits)))
        = mean_i [ logsumexp_k(w*clip(cos_ik)) - w*clip(cos_ii) ]   (b cancels)

Sharding: data-parallel over speakers N; 512 speakers (4 chunks of 128) per
core.

Key structure (v2 — PE-centric):
 - The m-sum (centroid numerator) rides the TensorEngine: 16 accumulating
   float32r matmuls per chunk against a bitcast identity (f32r streams at
   1 cycle/row for free-dim >= 256), instead of a DVE add-tree. This frees
   the DVE (the v1 bottleneck: 100% busy for 45us) and keeps the PE warm.
 - Centroids are normalized (rs_c) before the fp8 transpose+allgather.
   u is NOT normalized: rs_u is folded into the phase-C epilogue as
   per-partition tensor_scalar operands: y = max(ps*rs_u, eps).
 - Per-chunk fp8 AllGather fires as soon as that chunk's cT is ready
   (~2us after its load lands), pipelining the AG chain against the
   remaining loads. Bounce writes + AGs + gathered reads all ride the
   otherwise-empty gpsimd SWDGE ring so they are never queued behind the
   16.8MB of X-load descriptors on the sync HWDGE ring.
 - Phase-C matmul groups are interleaved into the PE FIFO in
   expected-data-arrival order (g0 after chunk2's m-sum, g1 split around
   chunk3's m-sum) so the PE never head-of-line blocks the AG-critical
   m-sums, yet fills its DMA-wait gaps with useful work.
 - Device ships per-(chunk, gather-group) exp-sums and the diagonal cos;
   host does s = sum(parts), rows = log(s) - w*clip(diag), mean.
"""

import os
import sys

for _p in ("/opt/trn_rl_repo",):
    if os.path.isdir(_p) and _p not in sys.path:
        sys.path.append(_p)

import numpy as np

import concourse.bass as bass
import concourse.tile as tile
from concourse import bacc, mybir
from concourse.bass_utils import run_bass_kernel_spmd
from concourse.masks import make_identity

N_CORES = 8
N, M, D = 4096, 16, 512
P = 128                     # partitions
LOCAL = N // N_CORES        # 512 speakers per core
NCHUNK = LOCAL // P         # 4 chunks of 128 speakers
NT = D // P                 # 4 d-tiles
EPS = 1e-6

F32 = mybir.dt.float32
F32R = mybir.dt.float32r
BF16 = mybir.dt.bfloat16
FP8 = mybir.dt.float8e4
AF = mybir.ActivationFunctionType
ALU = mybir.AluOpType


def build_program(w_val: float):
    nc = bacc.Bacc("TRN2", target_bir_lowering=False, debug=False,
                   num_devices=N_CORES)
    dvecs = nc.dram_tensor("dvecs", [LOCAL, M, D], F32, kind="ExternalInput").ap()
    out = nc.dram_tensor("out", [5, LOCAL], F32, kind="ExternalOutput").ap()

    with tile.TileContext(nc) as tc:
        _build(nc, tc, dvecs, out, w_val)
    nc.compile()
    return nc


def _build(nc, tc, dvecs, out, w_val):
    from contextlib import ExitStack
    ctx = ExitStack()
    with ctx:
        singles = ctx.enter_context(tc.tile_pool(name="singles", bufs=1))
        xpool = ctx.enter_context(tc.tile_pool(name="xpool", bufs=3))
        cpool = ctx.enter_context(tc.tile_pool(name="cpool", bufs=2))
        lpool = ctx.enter_context(tc.tile_pool(name="lpool", bufs=2))
        scr = ctx.enter_context(tc.tile_pool(name="scr", bufs=3))
        gpool = ctx.enter_context(tc.tile_pool(name="gpool", bufs=1))
        epool = ctx.enter_context(tc.tile_pool(name="epool", bufs=3))
        cpsum = ctx.enter_context(tc.tile_pool(name="cpsum", bufs=2, space="PSUM"))
        tpsum = ctx.enter_context(tc.tile_pool(name="tpsum", bufs=2, space="PSUM"))
        mpsum = ctx.enter_context(tc.tile_pool(name="mpsum", bufs=2, space="PSUM"))
        dram = ctx.enter_context(tc.tile_pool(name="dram", bufs=1, space="DRAM"))

        ident_bf = singles.tile([P, P], BF16)
        make_identity(nc, ident_bf)

        # persistent across the whole kernel
        uT = singles.tile([P, NT, LOCAL], BF16)       # u^T (raw): [d, t, i]
        ssq = singles.tile([P, NCHUNK, 2], F32)       # |c|^2, |u|^2
        nrm = singles.tile([P, NCHUNK, 2], F32)       # |c|, |u|
        rs = singles.tile([P, NCHUNK, 2], F32)        # 1/|c|, 1/|u|
        # stats[:, 0:4, q] = partial exp-sums per gather group
        # stats[:, 4, q]  = diag cos
        stats = singles.tile([P, 5, NCHUNK], F32)

        # Tiny dummy allgather fired immediately: the collective entry
        # barrier absorbs the 15-60us rank-start skew DURING the load
        # phase instead of delaying the first real allgather.
        warm_sb = singles.tile([P, 64], FP8)
        nc.gpsimd.memset(warm_sb, 0.0)
        warm_in = dram.tile([P, 64], FP8, name="warm_in")
        nc.gpsimd.dma_start(out=warm_in, in_=warm_sb)
        warm_out = dram.tile([N_CORES * P, 64], FP8, name="warm_out",
                             addr_space="Shared")
        nc.gpsimd.collective_compute(
            "AllGather", ALU.bypass,
            replica_groups=[list(range(N_CORES))],
            ins=[warm_in.opt()], outs=[warm_out.opt()],
        )

        # ---------- phase A: all X loads queue first on the sync ring ----
        xs = []
        for r in range(NCHUNK):
            x = xpool.tile([P, M, D], F32, name=f"x{r}", tag="x")
            if r == 0:
                # fine-grained so chunk0's m-sum can trail the DMA and its
                # allgather fires earliest
                for j in range(M // 2):
                    nc.sync.dma_start(
                        out=x[:, 2 * j:2 * j + 2, :],
                        in_=dvecs[0:P, 2 * j:2 * j + 2, :])
            else:
                nc.sync.dma_start(out=x, in_=dvecs[r * P:(r + 1) * P, :, :])
            xs.append(x)

        gath = [None] * NCHUNK
        g_sb = [None] * NCHUNK

        # ---------- phase B: per-chunk centroid pipeline + allgather -----
        def emit_chunk(r):
            x = xs[r]
            # m-sum: level-1 pair adds on DVE (f32 -> bf16 halves the data),
            # then 8 accumulating bf16 matmuls on the PE (bf16 streams at
            # 1 cyc/row and keeps the HAM clock-gate warm, unlike f32r
            # which measured ~2 cyc/row and never warms).
            l1 = lpool.tile([P, M // 2, D], BF16, name=f"l1_{r}", tag="l1")
            for j in range(M // 2):
                nc.vector.tensor_add(l1[:, j, :], x[:, 2 * j, :],
                                     x[:, 2 * j + 1, :])
            csum = cpsum.tile([P, D], F32, name=f"csum{r}", tag="csum")
            for j in range(M // 2):
                nc.tensor.matmul(csum, ident_bf, l1[:, j, :],
                                 start=(j == 0), stop=(j == M // 2 - 1))
            # norms: ssq via ACT Square+accum; sqrt; reciprocal on DVE
            sq_c = scr.tile([P, D], BF16, name=f"sqc{r}", tag="sq")
            nc.scalar.activation(sq_c, csum, AF.Square,
                                 accum_out=ssq[:, r, 0:1])
            sq_u = scr.tile([P, D], BF16, name=f"squ{r}", tag="sq")
            nc.scalar.activation(sq_u, x[:, M - 1, :], AF.Square,
                                 accum_out=ssq[:, r, 1:2])
            nc.scalar.activation(nrm[:, r, :], ssq[:, r, :], AF.Sqrt)
            nc.vector.reciprocal(rs[:, r, :], nrm[:, r, :])
            # normalized centroid + last utterance (bf16)
            chat = cpool.tile([P, D], BF16, name=f"chat{r}", tag="chat")
            nc.vector.tensor_scalar_mul(chat, csum, rs[:, r, 0:1])
            uhat = cpool.tile([P, D], BF16, name=f"uhat{r}", tag="uhat")
            nc.vector.tensor_scalar_mul(uhat, x[:, M - 1, :],
                                        rs[:, r, 1:2])
            # diag cos = sum_d chat*uhat
            dscr = scr.tile([P, D], BF16, name=f"dscr{r}", tag="sq")
            nc.vector.tensor_mul(dscr, chat, uhat)
            nc.vector.tensor_reduce(stats[:, 4, r:r + 1], dscr,
                                    axis=mybir.AxisListType.X,
                                    op=ALU.add)
            # uhat transposes -> one PSUM bank -> uT bf16 (single eviction)
            pu = tpsum.tile([P, NT, P], BF16, name=f"ptu{r}", tag="pt")
            for t in range(NT):
                nc.tensor.transpose(pu[:, t, :], uhat[:, t * P:(t + 1) * P],
                                    ident_bf)
            nc.scalar.copy(uT[:, :, r * P:(r + 1) * P], pu)
            # chat transposes (bf16) -> one PSUM bank -> cT fp8
            cT = cpool.tile([P, NT, P], FP8, name=f"cT{r}", tag="cT")
            pc = tpsum.tile([P, NT, P], BF16, name=f"ptc{r}", tag="pt")
            for t in range(NT):
                nc.tensor.transpose(pc[:, t, :], chat[:, t * P:(t + 1) * P],
                                    ident_bf)
            nc.scalar.copy(cT, pc)
            # bounce + allgather on the gpsimd SWDGE ring
            bounce = dram.tile([P, NT * P], FP8, name=f"bounce{r}")
            nc.gpsimd.dma_start(out=bounce,
                                in_=cT.rearrange("p t i -> p (t i)"))
            g = dram.tile([N_CORES * P, NT * P], FP8, name=f"gath{r}",
                          addr_space="Shared")
            nc.gpsimd.collective_compute(
                "AllGather", ALU.bypass,
                replica_groups=[list(range(N_CORES))],
                ins=[bounce.opt()], outs=[g.opt()],
            )
            gath[r] = g

        def emit_gread(gi):
            # gathered fp8 centroids -> SBUF [d, rank, t, i]. Rides the
            # scalar engine's HWDGE ring (qActDynamicHW): separate from the
            # sync ring so it is never queued behind the 16.8MB of X-load
            # descriptors, and HWDGE (not SWDGE) for the Shared-space read.
            g_sb[gi] = gpool.tile([P, N_CORES, NT, P], FP8, name=f"gsb{gi}",
                                  tag=f"gsb{gi}")
            nc.scalar.dma_start(
                out=g_sb[gi],
                in_=gath[gi].rearrange("(c p) f -> p c f", p=P).rearrange(
                    "p c (t i) -> p c t i", t=NT))

        def emit_cq(gi, q):
            # 512 queries x 1024 columns for gather group gi, query chunk q
            ps = mpsum.tile([P, 2, 512], F32, name=f"ps{gi}_{q}", tag="ps")
            for h in range(2):
                for t in range(NT):
                    nc.tensor.matmul(
                        ps[:, h, :],
                        uT[:, t, q * P:(q + 1) * P],
                        g_sb[gi][:, 4 * h:4 * h + 4, t, :],
                        start=(t == 0), stop=(t == NT - 1),
                    )
            # y = max(cos, eps); s_part = sum_k exp(w*y)
            y = epool.tile([P, 2 * 512], BF16, name=f"y{gi}_{q}", tag="y")
            nc.vector.tensor_scalar_max(y, ps.rearrange("p a b -> p (a b)"),
                                        EPS)
            e_scr = epool.tile([P, 2 * 512], BF16, name=f"e{gi}_{q}", tag="e")
            nc.scalar.activation(e_scr, y, AF.Exp, scale=w_val,
                                 accum_out=stats[:, gi, q:q + 1])

        # ALL phase-B work first on every engine FIFO, phase C strictly
        # after: kernel-start skew across ranks (15-45us observed) makes
        # gather completion times unpredictable, and any phase-B op queued
        # behind a gather-dependent op cascades into a late allgather.
        # With this ordering every bounce/AG fires on load cadence no
        # matter how late the collectives run.
        for r in range(NCHUNK):
            emit_chunk(r)
        for gi in range(NCHUNK):
            emit_gread(gi)
            for q in range(NCHUNK):
                emit_cq(gi, q)

        # ---------- ship results (per-row writes, baseline pattern) ------
        for k in range(5):
            nc.sync.dma_start(out=out[k].rearrange("(q p) -> p q", p=P),
                              in_=stats[:, k, :])


_CACHE = {}


def kernel(dvecs, w, b):
    w_val = float(np.asarray(w))
    key = w_val
    if key not in _CACHE:
        _CACHE[key] = build_program(w_val)
    nc = _CACHE[key]
    dvecs = np.ascontiguousarray(np.asarray(dvecs, dtype=np.float32))
    in_maps = [
        {"dvecs": dvecs[c * LOCAL:(c + 1) * LOCAL]} for c in range(N_CORES)
    ]
    res = run_bass_kernel_spmd(nc, in_maps, core_ids=list(range(N_CORES)))
    total = 0.0
    for c in range(N_CORES):
        o = np.asarray(res.results[c]["out"], dtype=np.float64)
        s = o[0:4].sum(axis=0)
        diag = o[4]
        rows = np.log(s) - w_val * np.maximum(diag, EPS)
        total += float(rows.sum())
    return np.float32(total / N)


# revision 25
# speedup vs baseline: 1.2176x; 1.2066x over previous
"""AngProtoLoss (stable) distributed Bass kernel for 8 TRN2 NeuronCores.

Problem (reference):
    dvecs: (4096, 16, 512) f32
    centroids = mean(dvecs, axis=1)                  # (N, D)
    u = dvecs[:, -1, :]                              # (N, D)
    cos = clip(cos_sim(u, centroids), min=1e-6)      # (N, N)
    logits = cos * w + b
    loss = -mean(diag(log_softmax(logits)))
        = mean_i [ logsumexp_k(w*clip(cos_ik)) - w*clip(cos_ii) ]   (b cancels)

Sharding: data-parallel over speakers N. Each core gets 512 speakers (4
chunks of 128), computes local normalized centroids (bf16 tree sum ->
rsqrt-normalize), transposes them on the TensorE, all-gathers them in fp8
(one allgather per chunk, pipelined against the load/centroid phase), then
computes its 512 rows of the cos matrix in bf16 x fp8 matmuls, applies
clip+exp (with ScalarE accumulate) for the log-sum-exp, and the local
diagonal terms. Device outputs per-row exp-sums and diagonal cos; the host
unshard does rows = log(s) - w*clip(diag) and means over N (b cancels in
log-softmax exactly).

Schedule notes (engine queues are FIFO):
 - gpsimd queue holds only the bounce writes + collectives so each allgather
   triggers as soon as its bounce is written (never stuck behind big loads).
 - X loads and gathered reads ride the sync HWDGE ring in data-ready order.
 - explicit add_dep edges keep phase-C matmuls/epilogue behind all phase-B
   work on PE/DVE/ACT queues, so a late chunk's transposes are never stalled
   behind ops waiting on a gather.
 - bounce layout is [128 d-rows x 512B (t,i)-cols] so every DMA touching
   HBM moves >=512B contiguous runs (small descriptors starve during
   collectives).
"""

import os
import sys

for _p in ("/opt/trn_rl_repo",):
    if os.path.isdir(_p) and _p not in sys.path:
        sys.path.append(_p)

import numpy as np

import concourse.bass as bass
import concourse.tile as tile
from concourse import bacc, mybir
from concourse.bass_utils import run_bass_kernel_spmd
from concourse.masks import make_identity

N_CORES = 8
N, M, D = 4096, 16, 512
P = 128                     # partitions
LOCAL = N // N_CORES        # 512 speakers per core
NCHUNK = LOCAL // P         # 4 chunks of 128 speakers
NT = D // P                 # 4 d-tiles
EPS = 1e-6

F32 = mybir.dt.float32
BF16 = mybir.dt.bfloat16
FP8 = mybir.dt.float8e4
AF = mybir.ActivationFunctionType


def build_program(w_val: float):
    nc = bacc.Bacc("TRN2", target_bir_lowering=False, debug=False,
                   num_devices=N_CORES)
    dvecs = nc.dram_tensor("dvecs", [LOCAL, M, D], F32, kind="ExternalInput").ap()
    out = nc.dram_tensor("out", [2, LOCAL], F32, kind="ExternalOutput").ap()

    with tile.TileContext(nc) as tc:
        _build(nc, tc, dvecs, out, w_val)
    nc.compile()
    return nc


def _build(nc, tc, dvecs, out, w_val):
    from contextlib import ExitStack
    ctx = ExitStack()
    with ctx:
        singles = ctx.enter_context(tc.tile_pool(name="singles", bufs=1))
        xpool = ctx.enter_context(tc.tile_pool(name="xpool", bufs=2))
        tree = ctx.enter_context(tc.tile_pool(name="tree", bufs=2))
        cpool = ctx.enter_context(tc.tile_pool(name="cpool", bufs=2))
        stats = ctx.enter_context(tc.tile_pool(name="stats", bufs=4))
        gpool = ctx.enter_context(tc.tile_pool(name="gpool", bufs=1))
        epool = ctx.enter_context(tc.tile_pool(name="epool", bufs=4))
        tpsum = ctx.enter_context(tc.tile_pool(name="tpsum", bufs=2, space="PSUM"))
        mpsum = ctx.enter_context(tc.tile_pool(name="mpsum", bufs=3, space="PSUM"))
        dram = ctx.enter_context(tc.tile_pool(name="dram", bufs=1, space="DRAM"))

        ident = singles.tile([P, P], F32)
        make_identity(nc, ident)

        # persistent across the whole kernel
        uT = singles.tile([P, NT, LOCAL], BF16)          # u^T: [d_in_tile, t, i]
        s_acc = singles.tile([P, NCHUNK], F32)           # sum_k exp(w*clip(cos))
        diag_all = singles.tile([P, NCHUNK], F32)        # diag cos, per q
        nc.vector.memset(s_acc, 0.0)

        # ---------- phase A: loads first (sync ring order) ----------
        xs = []
        for r in range(NCHUNK):
            x = xpool.tile([P, M, D], F32, name=f"x{r}", tag="x")
            nc.sync.dma_start(out=x, in_=dvecs[r * P:(r + 1) * P, :, :])
            xs.append(x)

        # ---------- phase B: per-chunk centroid pipeline + allgather ----------
        # One allgather per chunk (grouping chunks into fewer, bigger AGs
        # measured slower: the 1MB fp8 AG falls into the slow RDH regime).
        GROUPS = [[0], [1], [2], [3]]
        chunk_group = {}
        for gi, grp in enumerate(GROUPS):
            for slot, rr in enumerate(grp):
                chunk_group[rr] = (gi, slot)
        bounces = [None] * len(GROUPS)
        cc_insts = []
        gath = []
        last_transpose = [None]
        last_dve_b = [None]
        last_act_b = [None]
        # ssq/scale slots for all chunks: [:, r, 0] = centroid, [:, r, 1] = u.
        # Norm transcendentals (Ln then Exp) run batched per chunk PAIR so the
        # ACT table set switches ~4x per kernel instead of 14x -- each switch
        # is a 1.5us TDRAM DMA that lands inside the collective windows.
        ssq_all = singles.tile([P, NCHUNK, 2], F32)
        scales_all = singles.tile([P, NCHUNK, 2], F32)
        csums = []
        u_saves = []
        for r in range(NCHUNK):
            x = xs[r]
            # centroid sum over m: first level casts f32 -> bf16
            t1 = tree.tile([P, M // 2, D], BF16, name=f"t1_{r}", tag="t1")
            for j in range(M // 2):
                nc.vector.tensor_add(t1[:, j, :], x[:, 2 * j, :], x[:, 2 * j + 1, :])
            t2 = tree.tile([P, M // 4, D], BF16, name=f"t2_{r}", tag="t2")
            for j in range(M // 4):
                nc.vector.tensor_add(t2[:, j, :], t1[:, 2 * j, :], t1[:, 2 * j + 1, :])
            t3 = tree.tile([P, M // 8, D], BF16, name=f"t3_{r}", tag="t3")
            for j in range(M // 8):
                nc.vector.tensor_add(t3[:, j, :], t2[:, 2 * j, :], t2[:, 2 * j + 1, :])
            csum = cpool.tile([P, D], BF16, name=f"csum{r}", tag="csum")
            nc.vector.tensor_add(csum, t3[:, 0, :], t3[:, 1, :])
            csums.append(csum)

            # save the last utterance (frees the big X tile early)
            u_save = cpool.tile([P, D], BF16, name=f"usave{r}", tag="usave")
            nc.vector.tensor_copy(u_save, x[:, M - 1, :])
            u_saves.append(u_save)

            sq_scr = cpool.tile([P, D], BF16, name=f"sqscr{r}", tag="sqscr")
            nc.vector.tensor_mul(sq_scr, csum, csum)
            nc.vector.tensor_reduce(ssq_all[:, r, 0:1], sq_scr,
                                    axis=mybir.AxisListType.X,
                                    op=mybir.AluOpType.add)
            nc.vector.tensor_mul(sq_scr, x[:, M - 1, :], x[:, M - 1, :])
            nc.vector.tensor_reduce(ssq_all[:, r, 1:2], sq_scr,
                                    axis=mybir.AxisListType.X,
                                    op=mybir.AluOpType.add)

            if r % 2 == 0:
                continue
            # ---- batched norms + downstream for the pair (r-1, r) ----
            p0 = r - 1
            ln_scr = stats.tile([P, 2, 2], F32, name=f"ln{r}", tag="ln")
            nc.scalar.activation(ln_scr, ssq_all[:, p0:r + 1, :], AF.Ln)
            nc.scalar.activation(scales_all[:, p0:r + 1, :], ln_scr,
                                 AF.Exp, scale=-0.5)
            for rr in (p0, r):
                # normalize (f32 out: PSUM->SBUF copies ride ScalarE)
                chat = cpool.tile([P, D], F32, name=f"chat{rr}", tag="chat")
                uhat = cpool.tile([P, D], F32, name=f"uhat{rr}", tag="uhat")
                nc.vector.tensor_scalar_mul(chat, csums[rr],
                                            scales_all[:, rr, 0:1])
                nc.vector.tensor_scalar_mul(uhat, u_saves[rr],
                                            scales_all[:, rr, 1:2])

                # diagonal cos (local)
                dg_scr = cpool.tile([P, D], F32, name=f"dgscr{rr}", tag="dgscr")
                nc.vector.tensor_mul(dg_scr, chat, uhat)
                rd = nc.vector.tensor_reduce(diag_all[:, rr:rr + 1], dg_scr,
                                             axis=mybir.AxisListType.X,
                                             op=mybir.AluOpType.add)
                last_dve_b[0] = rd.ins

                # transposes on PE (f32 -> f32 psum), cast to fp8/bf16 on ACT
                cT = cpool.tile([P, NT, P], FP8, name=f"cT{rr}", tag="cT")
                for t in range(NT):
                    pt = tpsum.tile([P, P], F32, name=f"ptc{rr}_{t}", tag="pt")
                    ti = nc.tensor.transpose(pt, chat[:, t * P:(t + 1) * P],
                                             ident)
                    last_transpose[0] = ti.ins
                    nc.scalar.copy(cT[:, t, :], pt)
                    pu = tpsum.tile([P, P], F32, name=f"ptu{rr}_{t}", tag="pt")
                    ti = nc.tensor.transpose(pu, uhat[:, t * P:(t + 1) * P],
                                             ident)
                    last_transpose[0] = ti.ins
                    cp = nc.scalar.copy(uT[:, t, rr * P:(rr + 1) * P], pu)
                    last_act_b[0] = cp.ins

                # bounce write (fp8) on the gpsimd SWDGE ring (otherwise
                # empty) so it is not FIFO-serialized behind the X loads.
                gi, slot = chunk_group[rr]
                L = len(GROUPS[gi])
                bounces[gi] = bounces[gi] if bounces[gi] is not None else \
                    dram.tile([L * P, NT * P], FP8, name=f"bounce_g{gi}")
                nc.gpsimd.dma_start(
                    out=bounces[gi][slot * P:(slot + 1) * P, :],
                    in_=cT.rearrange("p t i -> p (t i)"))
                if slot == L - 1:
                    g = dram.tile([N_CORES * L * P, NT * P], FP8,
                                  name=f"gath{gi}", addr_space="Shared")
                    cc = nc.gpsimd.collective_compute(
                        "AllGather", mybir.AluOpType.bypass,
                        replica_groups=[list(range(N_CORES))],
                        ins=[bounces[gi].opt()], outs=[g.opt()],
                    )
                    cc_insts.append(cc.ins)
                    gath.append(g)

        # diag rows are complete after phase B: ship them now, off the tail
        nc.sync.dma_start(out=out[1].rearrange("(q p) -> p q", p=P),
                          in_=diag_all)

        # ---------- phase C: gathered reads + matmuls + epilogue ----------
        # Ordering guards: phase-C work on DVE/ACT must sit behind all
        # phase-B work on those queues, so late chunks are never stalled
        # behind epilogue ops waiting on a gather.
        last_dve = [last_dve_b[0]]
        last_act = [last_act_b[0]]
        for gi, grp in enumerate(GROUPS):
            L = len(grp)
            g_sb = gpool.tile([P, N_CORES * L, NT, P], FP8, name=f"gsb{gi}",
                              tag=f"gsb{gi}")
            nc.sync.dma_start(
                out=g_sb,
                in_=gath[gi].rearrange("(c p) f -> p c f", p=P).rearrange(
                    "p c (t i) -> p c t i", t=NT))
            for q in range(NCHUNK):
                for slot in range(L):
                    ps = mpsum.tile([P, 2, N_CORES // 2 * P], F32,
                                    name=f"ps{gi}_{q}_{slot}", tag="ps")
                    for h in range(2):
                        for t in range(NT):
                            # rhs: ranks c in [4h,4h+4), chunk slot, d-tile t
                            cs = 4 * h * L + slot
                            rhs = g_sb[:, cs:cs + 3 * L + 1:L, t, :]
                            mm = nc.tensor.matmul(
                                ps[:, h, :],
                                uT[:, t, q * P:(q + 1) * P],
                                rhs,
                                start=(t == 0), stop=(t == NT - 1),
                            )
                            # keep every matmul behind all transposes in the
                            # PE queue so late-chunk transposes are never
                            # stalled by matmuls waiting on a gather.
                            if h == 0 and t == 0:
                                tile.add_dep_helper(
                                    mm.ins, last_transpose[0], sync=True,
                                    reason="PE: transposes before matmuls")
                    # epilogue: y = max(cos, eps); s += sum_k exp(w*y)
                    y = epool.tile([P, 2 * (N_CORES // 2) * P], BF16,
                                   name=f"y{gi}_{q}_{slot}", tag="y")
                    mx = nc.vector.tensor_scalar_max(
                        y, ps.rearrange("p a b -> p (a b)"), EPS)
                    e_scr = epool.tile([P, 2 * (N_CORES // 2) * P], BF16,
                                       name=f"escr{gi}_{q}_{slot}", tag="escr")
                    s_part = stats.tile([P, 1], F32, name=f"sp{gi}_{q}_{slot}",
                                        tag="sp")
                    ex = nc.scalar.activation(e_scr, y, AF.Exp, scale=w_val,
                                              accum_out=s_part)
                    nc.vector.tensor_add(s_acc[:, q:q + 1], s_acc[:, q:q + 1],
                                         s_part)

        # ---------- finals: ship s (exp-sums); host does the log.
        # (diag was already shipped right after phase B, off the tail.)
        nc.sync.dma_start(out=out[0].rearrange("(q p) -> p q", p=P), in_=s_acc)


_CACHE = {}


def kernel(dvecs, w, b):
    w_val = float(np.asarray(w))
    key = w_val
    if key not in _CACHE:
        _CACHE[key] = build_program(w_val)
    nc = _CACHE[key]
    dvecs = np.ascontiguousarray(np.asarray(dvecs, dtype=np.float32))
    in_maps = [
        {"dvecs": dvecs[c * LOCAL:(c + 1) * LOCAL]} for c in range(N_CORES)
    ]
    res = run_bass_kernel_spmd(nc, in_maps, core_ids=list(range(N_CORES)))
    total = 0.0
    for c in range(N_CORES):
        o = np.asarray(res.results[c]["out"], dtype=np.float64)
        s, diag = o[0], o[1]
        rows = np.log(s) - w_val * np.maximum(diag, EPS)
        total += float(rows.sum())
    return np.float32(total / N)



# revision 26
# speedup vs baseline: 1.2547x; 1.0305x over previous
"""AngProtoLoss (stable) distributed Bass kernel for 8 TRN2 NeuronCores.

Problem (reference):
    dvecs: (4096, 16, 512) f32
    centroids = mean(dvecs, axis=1)                  # (N, D)
    u = dvecs[:, -1, :]                              # (N, D)
    cos = clip(cos_sim(u, centroids), min=1e-6)      # (N, N)
    logits = cos * w + b
    loss = -mean(diag(log_softmax(logits)))
        = mean_i [ logsumexp_k(w*clip(cos_ik)) - w*clip(cos_ii) ]   (b cancels)

Sharding: data-parallel over speakers N. Each core gets 512 speakers (4
chunks of 128), computes local normalized centroids (bf16 tree sum ->
rsqrt-normalize), transposes them on the TensorE, all-gathers them in fp8
(one allgather per chunk, pipelined against the load/centroid phase), then
computes its 512 rows of the cos matrix in bf16 x fp8 matmuls, applies
clip+exp (with ScalarE accumulate) for the log-sum-exp, and the local
diagonal terms. Device outputs per-row exp-sums and diagonal cos; the host
unshard does rows = log(s) - w*clip(diag) and means over N (b cancels in
log-softmax exactly).

Schedule notes (engine queues are FIFO):
 - gpsimd queue holds only the bounce writes + collectives so each allgather
   triggers as soon as its bounce is written (never stuck behind big loads).
 - X loads and gathered reads ride the sync HWDGE ring in data-ready order.
 - explicit add_dep edges keep phase-C matmuls/epilogue behind all phase-B
   work on PE/DVE/ACT queues, so a late chunk's transposes are never stalled
   behind ops waiting on a gather.
 - bounce layout is [128 d-rows x 512B (t,i)-cols] so every DMA touching
   HBM moves >=512B contiguous runs (small descriptors starve during
   collectives).
"""

import os
import sys

for _p in ("/opt/trn_rl_repo",):
    if os.path.isdir(_p) and _p not in sys.path:
        sys.path.append(_p)

import numpy as np

import concourse.bass as bass
import concourse.tile as tile
from concourse import bacc, mybir
from concourse.bass_utils import run_bass_kernel_spmd
from concourse.masks import make_identity

N_CORES = 8
N, M, D = 4096, 16, 512
P = 128                     # partitions
LOCAL = N // N_CORES        # 512 speakers per core
NCHUNK = LOCAL // P         # 4 chunks of 128 speakers
NT = D // P                 # 4 d-tiles
EPS = 1e-6

F32 = mybir.dt.float32
BF16 = mybir.dt.bfloat16
FP8 = mybir.dt.float8e4
AF = mybir.ActivationFunctionType


def build_program(w_val: float):
    nc = bacc.Bacc("TRN2", target_bir_lowering=False, debug=False,
                   num_devices=N_CORES)
    dvecs = nc.dram_tensor("dvecs", [LOCAL, M, D], F32, kind="ExternalInput").ap()
    out = nc.dram_tensor("out", [2, LOCAL], F32, kind="ExternalOutput").ap()

    with tile.TileContext(nc) as tc:
        _build(nc, tc, dvecs, out, w_val)
    nc.compile()
    return nc


def _build(nc, tc, dvecs, out, w_val):
    from contextlib import ExitStack
    ctx = ExitStack()
    with ctx:
        singles = ctx.enter_context(tc.tile_pool(name="singles", bufs=1))
        xpool = ctx.enter_context(tc.tile_pool(name="xpool", bufs=2))
        tree = ctx.enter_context(tc.tile_pool(name="tree", bufs=2))
        cpool = ctx.enter_context(tc.tile_pool(name="cpool", bufs=2))
        stats = ctx.enter_context(tc.tile_pool(name="stats", bufs=4))
        gpool = ctx.enter_context(tc.tile_pool(name="gpool", bufs=1))
        epool = ctx.enter_context(tc.tile_pool(name="epool", bufs=4))
        tpsum = ctx.enter_context(tc.tile_pool(name="tpsum", bufs=2, space="PSUM"))
        mpsum = ctx.enter_context(tc.tile_pool(name="mpsum", bufs=2, space="PSUM"))
        cpsum = ctx.enter_context(tc.tile_pool(name="cpsum", bufs=2, space="PSUM"))
        dram = ctx.enter_context(tc.tile_pool(name="dram", bufs=1, space="DRAM"))

        ident = singles.tile([P, P], F32)
        make_identity(nc, ident)
        ident_bf = singles.tile([P, P], BF16)
        make_identity(nc, ident_bf)

        # persistent across the whole kernel
        uT = singles.tile([P, NT, LOCAL], BF16)          # u^T: [d_in_tile, t, i]
        s_acc = singles.tile([P, NCHUNK], F32)           # sum_k exp(w*clip(cos))
        diag_all = singles.tile([P, NCHUNK], F32)        # diag cos, per q
        nc.vector.memset(s_acc, 0.0)

        # ---------- phase A: loads first (sync ring order) ----------
        xs = []
        for r in range(NCHUNK):
            x = xpool.tile([P, M, D], F32, name=f"x{r}", tag="x")
            nc.sync.dma_start(out=x, in_=dvecs[r * P:(r + 1) * P, :, :])
            xs.append(x)

        # ---------- phase B: per-chunk centroid pipeline + allgather ----------
        # One allgather per chunk (grouping chunks into fewer, bigger AGs
        # measured slower: the 1MB fp8 AG falls into the slow RDH regime).
        GROUPS = [[0], [1], [2], [3]]
        chunk_group = {}
        for gi, grp in enumerate(GROUPS):
            for slot, rr in enumerate(grp):
                chunk_group[rr] = (gi, slot)
        bounces = [None] * len(GROUPS)
        cc_insts = []
        gath = []
        last_transpose = [None]
        last_dve_b = [None]
        last_act_b = [None]
        # ssq/scale slots for all chunks: [:, r, 0] = centroid, [:, r, 1] = u.
        # Norm transcendentals (Ln then Exp) run batched per chunk PAIR so the
        # ACT table set switches ~4x per kernel instead of 14x -- each switch
        # is a 1.5us TDRAM DMA that lands inside the collective windows.
        ssq_all = singles.tile([P, NCHUNK, 2], F32)
        scales_all = singles.tile([P, NCHUNK, 2], F32)
        csums = []
        u_saves = []
        for r in range(NCHUNK):
            x = xs[r]
            # centroid sum over m: first level casts f32 -> bf16
            t1 = tree.tile([P, M // 2, D], BF16, name=f"t1_{r}", tag="t1")
            for j in range(M // 2):
                nc.vector.tensor_add(t1[:, j, :], x[:, 2 * j, :], x[:, 2 * j + 1, :])
            # levels 2-4 ride the TensorEngine: 8 accumulating bf16 matmuls
            # against the identity (frees ~11us of critical-phase DVE time)
            csum_ps = cpsum.tile([P, D], F32, name=f"csump{r}", tag="csump")
            for j in range(M // 2):
                nc.tensor.matmul(csum_ps, ident_bf, t1[:, j, :],
                                 start=(j == 0), stop=(j == M // 2 - 1))
            csum = cpool.tile([P, D], BF16, name=f"csum{r}", tag="csum")
            nc.scalar.copy(csum, csum_ps)
            csums.append(csum)

            # save the last utterance (frees the big X tile early)
            u_save = cpool.tile([P, D], BF16, name=f"usave{r}", tag="usave")
            nc.vector.tensor_copy(u_save, x[:, M - 1, :])
            u_saves.append(u_save)

            sq_scr = cpool.tile([P, D], BF16, name=f"sqscr{r}", tag="sqscr")
            nc.vector.tensor_mul(sq_scr, csum, csum)
            nc.vector.tensor_reduce(ssq_all[:, r, 0:1], sq_scr,
                                    axis=mybir.AxisListType.X,
                                    op=mybir.AluOpType.add)
            nc.vector.tensor_mul(sq_scr, x[:, M - 1, :], x[:, M - 1, :])
            nc.vector.tensor_reduce(ssq_all[:, r, 1:2], sq_scr,
                                    axis=mybir.AxisListType.X,
                                    op=mybir.AluOpType.add)

            if r % 2 == 0:
                continue
            # ---- batched norms + downstream for the pair (r-1, r) ----
            p0 = r - 1
            ln_scr = stats.tile([P, 2, 2], F32, name=f"ln{r}", tag="ln")
            nc.scalar.activation(ln_scr, ssq_all[:, p0:r + 1, :], AF.Ln)
            nc.scalar.activation(scales_all[:, p0:r + 1, :], ln_scr,
                                 AF.Exp, scale=-0.5)
            for rr in (p0, r):
                # normalize (f32 out: PSUM->SBUF copies ride ScalarE)
                chat = cpool.tile([P, D], F32, name=f"chat{rr}", tag="chat")
                uhat = cpool.tile([P, D], F32, name=f"uhat{rr}", tag="uhat")
                nc.vector.tensor_scalar_mul(chat, csums[rr],
                                            scales_all[:, rr, 0:1])
                nc.vector.tensor_scalar_mul(uhat, u_saves[rr],
                                            scales_all[:, rr, 1:2])

                # diagonal cos (local)
                dg_scr = cpool.tile([P, D], F32, name=f"dgscr{rr}", tag="dgscr")
                nc.vector.tensor_mul(dg_scr, chat, uhat)
                rd = nc.vector.tensor_reduce(diag_all[:, rr:rr + 1], dg_scr,
                                             axis=mybir.AxisListType.X,
                                             op=mybir.AluOpType.add)
                last_dve_b[0] = rd.ins

                # transposes on PE (f32 -> f32 psum), cast to fp8/bf16 on ACT
                cT = cpool.tile([P, NT, P], FP8, name=f"cT{rr}", tag="cT")
                for t in range(NT):
                    pt = tpsum.tile([P, P], F32, name=f"ptc{rr}_{t}", tag="pt")
                    ti = nc.tensor.transpose(pt, chat[:, t * P:(t + 1) * P],
                                             ident)
                    last_transpose[0] = ti.ins
                    nc.scalar.copy(cT[:, t, :], pt)
                    pu = tpsum.tile([P, P], F32, name=f"ptu{rr}_{t}", tag="pt")
                    ti = nc.tensor.transpose(pu, uhat[:, t * P:(t + 1) * P],
                                             ident)
                    last_transpose[0] = ti.ins
                    cp = nc.scalar.copy(uT[:, t, rr * P:(rr + 1) * P], pu)
                    last_act_b[0] = cp.ins

                # bounce write (fp8) on the gpsimd SWDGE ring (otherwise
                # empty) so it is not FIFO-serialized behind the X loads.
                gi, slot = chunk_group[rr]
                L = len(GROUPS[gi])
                bounces[gi] = bounces[gi] if bounces[gi] is not None else \
                    dram.tile([L * P, NT * P], FP8, name=f"bounce_g{gi}")
                nc.gpsimd.dma_start(
                    out=bounces[gi][slot * P:(slot + 1) * P, :],
                    in_=cT.rearrange("p t i -> p (t i)"))
                if slot == L - 1:
                    g = dram.tile([N_CORES * L * P, NT * P], FP8,
                                  name=f"gath{gi}", addr_space="Shared")
                    cc = nc.gpsimd.collective_compute(
                        "AllGather", mybir.AluOpType.bypass,
                        replica_groups=[list(range(N_CORES))],
                        ins=[bounces[gi].opt()], outs=[g.opt()],
                    )
                    cc_insts.append(cc.ins)
                    gath.append(g)

        # diag rows are complete after phase B: ship them now, off the tail
        nc.sync.dma_start(out=out[1].rearrange("(q p) -> p q", p=P),
                          in_=diag_all)

        # ---------- phase C: gathered reads + matmuls + epilogue ----------
        # Ordering guards: phase-C work on DVE/ACT must sit behind all
        # phase-B work on those queues, so late chunks are never stalled
        # behind epilogue ops waiting on a gather.
        last_dve = [last_dve_b[0]]
        last_act = [last_act_b[0]]
        for gi, grp in enumerate(GROUPS):
            L = len(grp)
            g_sb = gpool.tile([P, N_CORES * L, NT, P], FP8, name=f"gsb{gi}",
                              tag=f"gsb{gi}")
            nc.sync.dma_start(
                out=g_sb,
                in_=gath[gi].rearrange("(c p) f -> p c f", p=P).rearrange(
                    "p c (t i) -> p c t i", t=NT))
            for q in range(NCHUNK):
                for slot in range(L):
                    ps = mpsum.tile([P, 2, N_CORES // 2 * P], F32,
                                    name=f"ps{gi}_{q}_{slot}", tag="ps")
                    for h in range(2):
                        for t in range(NT):
                            # rhs: ranks c in [4h,4h+4), chunk slot, d-tile t
                            cs = 4 * h * L + slot
                            rhs = g_sb[:, cs:cs + 3 * L + 1:L, t, :]
                            mm = nc.tensor.matmul(
                                ps[:, h, :],
                                uT[:, t, q * P:(q + 1) * P],
                                rhs,
                                start=(t == 0), stop=(t == NT - 1),
                            )
                            # keep every matmul behind all transposes in the
                            # PE queue so late-chunk transposes are never
                            # stalled by matmuls waiting on a gather.
                            if h == 0 and t == 0:
                                tile.add_dep_helper(
                                    mm.ins, last_transpose[0], sync=True,
                                    reason="PE: transposes before matmuls")
                    # epilogue: y = max(cos, eps); s += sum_k exp(w*y)
                    y = epool.tile([P, 2 * (N_CORES // 2) * P], BF16,
                                   name=f"y{gi}_{q}_{slot}", tag="y")
                    mx = nc.vector.tensor_scalar_max(
                        y, ps.rearrange("p a b -> p (a b)"), EPS)
                    e_scr = epool.tile([P, 2 * (N_CORES // 2) * P], BF16,
                                       name=f"escr{gi}_{q}_{slot}", tag="escr")
                    s_part = stats.tile([P, 1], F32, name=f"sp{gi}_{q}_{slot}",
                                        tag="sp")
                    ex = nc.scalar.activation(e_scr, y, AF.Exp, scale=w_val,
                                              accum_out=s_part)
                    nc.vector.tensor_add(s_acc[:, q:q + 1], s_acc[:, q:q + 1],
                                         s_part)

        # ---------- finals: ship s (exp-sums); host does the log.
        # (diag was already shipped right after phase B, off the tail.)
        nc.sync.dma_start(out=out[0].rearrange("(q p) -> p q", p=P), in_=s_acc)


_CACHE = {}


def kernel(dvecs, w, b):
    w_val = float(np.asarray(w))
    key = w_val
    if key not in _CACHE:
        _CACHE[key] = build_program(w_val)
    nc = _CACHE[key]
    dvecs = np.ascontiguousarray(np.asarray(dvecs, dtype=np.float32))
    in_maps = [
        {"dvecs": dvecs[c * LOCAL:(c + 1) * LOCAL]} for c in range(N_CORES)
    ]
    res = run_bass_kernel_spmd(nc, in_maps, core_ids=list(range(N_CORES)))
    total = 0.0
    for c in range(N_CORES):
        o = np.asarray(res.results[c]["out"], dtype=np.float64)
        s, diag = o[0], o[1]
        rows = np.log(s) - w_val * np.maximum(diag, EPS)
        total += float(rows.sum())
    return np.float32(total / N)

